# revision 1
# baseline (speedup 1.0000x reference)
"""Trainium2 Bass kernel for GatedCrossAttention (B=4, N=4096, C=1024, H=16, M=4).

Reference math (dead code removed: the v/gate projections are overwritten
by views of k in the original module, so v = g = k):
    q = query @ Wq.T + bq                    [B,N,C]   -> [B,N,H,hd]
    k = key   @ Wk.T + bk                    [B,N,M,C] -> [B,N,M,H,hd]
    attn = softmax_M(SCALE * einsum('bnhc,bnmhc->bnmh', q, k))
    out  = einsum('bnmh,bnmhc->bnhc', attn, k*k) . reshape(B,N,C)
    out  = out @ Wo.T + bo

Strategy: pure data parallel over the 16384 tokens (8 cores x 2048), no
collectives.  On-chip layout is "transposed": channels on partitions, tokens
on the free axis, so every matmul contraction (over channels) is a natural
PE op.  The per-head segment reductions use indicator matmuls with the
elementwise product q*k as the stationary operand, landing logits token-major
([t, (m,h)]) so the M-softmax runs on all 128 partitions; a tiny PE transpose
of the softmax weights returns them head-major for the head->channel
broadcast matmuls.  Host pre-transposes/casts inputs and weights to fp16
(error vs f32 reference ~1e-3, PE runs 16-bit at full rate), accumulation
stays f32 in PSUM.
"""

import dataclasses
import numpy as np
from contextlib import ExitStack

try:
    import concourse.bass as bass
except ImportError:  # path fallback for bare containers
    import sys

    sys.path.insert(0, "/opt/trn_rl_repo")
    import concourse.bass as bass

import concourse.tile as tile
from concourse import bacc, mybir
from concourse.bass_utils import run_bass_kernel_spmd
from concourse.masks import make_identity

# problem constants (hardcoded per the task contract)
B, N, C, H, HD, M = 4, 4096, 1024, 16, 64, 4
SCALE = float(HD) ** -0.5
NCORES = 8
T_TOTAL = B * N
T_CORE = T_TOTAL // NCORES  # 2048
TB = 512                    # tokens per block (one PSUM bank at f32)
NJ = C // 128               # 8 channel tiles
NT = TB // 128              # 4 token subtiles per block

DT = mybir.dt.float16
NPDT = np.float16
F32 = mybir.dt.float32


def _bcast(ap, reps, axis):
    """Insert a 0-stride dim of size `reps` at AP position `axis` (0=partition)."""
    new = list(ap.ap)
    new.insert(axis, [0, reps])
    return dataclasses.replace(ap, ap=new)


def build_nc(t_core=T_CORE, with_bias=False):
    nblk = t_core // TB
    nc = bacc.Bacc("TRN2", target_bir_lowering=False, debug=False)

    qT = nc.declare_dram_parameter("qT", [C, t_core], DT, isOutput=False)
    kT = nc.declare_dram_parameter("kT", [M, C, t_core], DT, isOutput=False)
    wqT = nc.declare_dram_parameter("wqT", [C, C], DT, isOutput=False)
    wkT = nc.declare_dram_parameter("wkT", [C, C], DT, isOutput=False)
    woT = nc.declare_dram_parameter("woT", [C, C], DT, isOutput=False)
    indl = nc.declare_dram_parameter("indl", [128, 2], DT, isOutput=False)
    indb = nc.declare_dram_parameter("indb", [M, NJ, 64, 128], DT, isOutput=False)
    if with_bias:
        bq = nc.declare_dram_parameter("bq", [1, C], DT, isOutput=False)
        bk = nc.declare_dram_parameter("bk", [1, C], DT, isOutput=False)
        bo = nc.declare_dram_parameter("bo", [1, C], DT, isOutput=False)
    out = nc.declare_dram_parameter("out", [t_core, C], F32, isOutput=True)

    # DRAM views: channel dim split into (chunk, partition)
    qT_v = qT.ap().rearrange("(c p) t -> p c t", p=128)
    kT_v = kT.ap().rearrange("m (c p) t -> p m c t", p=128)
    wq_v = wqT.ap().rearrange("(c p) j -> p c j", p=128)
    wk_v = wkT.ap().rearrange("(c p) j -> p c j", p=128)
    wo_v = woT.ap().rearrange("(c p) j -> p c j", p=128)

    with tile.TileContext(nc) as tc, ExitStack() as ctx:
        consts = ctx.enter_context(tc.tile_pool(name="consts", bufs=1))
        p_inq = ctx.enter_context(tc.tile_pool(name="inq", bufs=2))
        p_ink = ctx.enter_context(tc.tile_pool(name="ink", bufs=2))
        p_qp = ctx.enter_context(tc.tile_pool(name="qp", bufs=9))
        p_kp = ctx.enter_context(tc.tile_pool(name="kp", bufs=8))
        p_prod = ctx.enter_context(tc.tile_pool(name="prod", bufs=2))
        p_ksq = ctx.enter_context(tc.tile_pool(name="ksq", bufs=2))
        p_sm = ctx.enter_context(tc.tile_pool(name="sm", bufs=2))
        p_ct = ctx.enter_context(tc.tile_pool(name="ct", bufs=2))
        p_y = ctx.enter_context(tc.tile_pool(name="y", bufs=2))
        p_yb = ctx.enter_context(tc.tile_pool(name="yb", bufs=10))
        p_out = ctx.enter_context(tc.tile_pool(name="outs", bufs=3))
        pp = ctx.enter_context(tc.tile_pool(name="pp", bufs=2, space="PSUM"))
        pl = ctx.enter_context(tc.tile_pool(name="pl", bufs=2, space="PSUM"))
        pw = ctx.enter_context(tc.tile_pool(name="pw", bufs=2, space="PSUM"))
        pb = ctx.enter_context(tc.tile_pool(name="pb", bufs=2, space="PSUM"))

        # ---- constants / weights (resident) ----
        wq_sb = consts.tile([128, NJ, C], DT)
        wk_sb = consts.tile([128, NJ, C], DT)
        wo_sb = consts.tile([128, NJ, C], DT)
        nc.sync.dma_start(out=wq_sb, in_=wq_v)
        nc.sync.dma_start(out=wk_sb, in_=wk_v)
        nc.sync.dma_start(out=wo_sb, in_=wo_v)
        indl_sb = consts.tile([128, 2], DT)
        indb_sb = consts.tile([64, M, NJ, 128], DT)
        nc.sync.dma_start(out=indl_sb, in_=indl.ap())
        nc.sync.dma_start(out=indb_sb, in_=indb.ap().rearrange("m r p j -> p m r j"))
        ident = consts.tile([128, 128], DT)
        make_identity(nc, ident)
        if with_bias:
            ones_sb = consts.tile([1, TB], DT)
            nc.vector.memset(ones_sb, 1.0)
            bq_sb = consts.tile([1, C], DT)
            bk_sb = consts.tile([1, C], DT)
            bo_sb = consts.tile([1, C], DT)
            nc.sync.dma_start(out=bq_sb, in_=bq.ap())
            nc.sync.dma_start(out=bk_sb, in_=bk.ap())
            nc.sync.dma_start(out=bo_sb, in_=bo.ap())

        for blk in range(nblk):
            t0 = blk * TB
            tsl = slice(t0, t0 + TB)

            # ---- load inputs ----
            q_in = p_inq.tile([128, NJ, TB], DT)
            nc.sync.dma_start(out=q_in, in_=qT_v[:, :, tsl])
            k_in = [p_ink.tile([128, NJ, TB], DT, tag="kin", name="kin")
                    for _ in range(M)]
            for m in range(M):
                nc.sync.dma_start(out=k_in[m], in_=kT_v[:, m, :, tsl])

            # ---- projections (PE) ----
            qp = [p_qp.tile([128, TB], DT, tag="qp", name="qp") for _ in range(NJ)]
            for r in range(NJ):
                ps = pp.tile([128, TB], F32, tag="pp", name="pp")
                for c in range(NJ):
                    nc.tensor.matmul(
                        ps,
                        wq_sb[:, c, r * 128:(r + 1) * 128],
                        q_in[:, c, :],
                        start=(c == 0),
                        stop=(c == NJ - 1 and not with_bias),
                    )
                if with_bias:
                    nc.tensor.matmul(
                        ps, bq_sb[:, r * 128:(r + 1) * 128], ones_sb,
                        start=False, stop=True,
                    )
                nc.scalar.copy(out=qp[r], in_=ps)

            kp = [p_kp.tile([128, M, TB], DT, tag="kp", name="kp")
                  for _ in range(NJ)]
            for m in range(M):
                for r in range(NJ):
                    ps = pp.tile([128, TB], F32, tag="pp", name="pp")
                    for c in range(NJ):
                        nc.tensor.matmul(
                            ps,
                            wk_sb[:, c, r * 128:(r + 1) * 128],
                            k_in[m][:, c, :],
                            start=(c == 0),
                            stop=(c == NJ - 1 and not with_bias),
                        )
                    if with_bias:
                        nc.tensor.matmul(
                            ps, bk_sb[:, r * 128:(r + 1) * 128], ones_sb,
                            start=False, stop=True,
                        )
                    nc.scalar.copy(out=kp[r][:, m, :], in_=ps)

            # ---- attention logits, token-major: pslt[t, tt, m, h] ----
            pslt = pl.tile([128, NT, M, H], F32, tag="pl", name="pl")
            for r in range(NJ):
                prod = p_prod.tile([128, M, TB], DT, tag="prod", name="prod")
                nc.vector.tensor_mul(prod, _bcast(qp[r], M, 1), kp[r])
                for tt in range(NT):
                    for m in range(M):
                        nc.tensor.matmul(
                            pslt[:, tt, m, 2 * r:2 * r + 2],
                            prod[:, m, tt * 128:(tt + 1) * 128],
                            indl_sb,
                            start=True,
                            stop=True,
                        )

            # ---- softmax over M (token-major, full 128 partitions) ----
            e = p_sm.tile([128, NT, M, H], F32, tag="e", name="e")
            nc.scalar.activation(e, pslt, func=mybir.ActivationFunctionType.Exp)
            s01 = p_sm.tile([128, NT, H], F32, tag="s01", name="s01")
            s = p_sm.tile([128, NT, H], F32, tag="s", name="s")
            nc.vector.tensor_add(s01, e[:, :, 0, :], e[:, :, 1, :])
            nc.vector.tensor_add(s, e[:, :, 2, :], e[:, :, 3, :])
            nc.vector.tensor_add(s, s01, s)
            rcp = p_sm.tile([128, NT, H], F32, tag="rcp", name="rcp")
            nc.vector.reciprocal(rcp, s)
            w_t = p_sm.tile([128, NT, M, H], DT, tag="w", name="w")
            nc.vector.tensor_mul(w_t, e, _bcast(rcp, M, 2))

            # transpose w to head-major: wT[(m,h), (tt,t)]
            wT = p_sm.tile([64, NT, 128], DT, tag="wT", name="wT")
            for tt in range(NT):
                pst = pw.tile([64, 128], DT, tag="pw", name="pw")
                nc.tensor.transpose(pst, w_t[:, tt, :, :], ident)
                nc.scalar.copy(out=wT[:, tt, :], in_=pst)

            # ---- weighted sum of k^2 (PE broadcast + DVE) ----
            yb = [p_yb.tile([128, TB], DT, tag="yb", name="yb") for _ in range(NJ)]
            for r in range(NJ):
                ksq = p_ksq.tile([128, M, TB], DT, tag="ksq", name="ksq")
                nc.vector.tensor_mul(ksq, kp[r], kp[r])
                ct = p_ct.tile([128, M, TB], F32, tag="ct", name="ct")
                for mp in range(2):      # m-pairs
                    for hf in range(2):  # half-blocks of 256 tokens
                        psb = pb.tile([128, 2, 256], F32, tag="pb", name="pb")
                        for mi in range(2):
                            m = 2 * mp + mi
                            for ti in range(2):
                                tt = 2 * hf + ti
                                nc.tensor.matmul(
                                    psb[:, mi, ti * 128:(ti + 1) * 128],
                                    indb_sb[:, m, r, :],
                                    wT[:, tt, :],
                                    start=True,
                                    stop=True,
                                )
                        nc.vector.tensor_mul(
                            ct[:, 2 * mp:2 * mp + 2, hf * 256:(hf + 1) * 256],
                            psb,
                            ksq[:, 2 * mp:2 * mp + 2, hf * 256:(hf + 1) * 256],
                        )
                y = p_y.tile([128, TB], F32, tag="y", name="y")
                nc.vector.reduce_sum(
                    y, ct.rearrange("p m t -> p t m"), axis=mybir.AxisListType.X
                )
                nc.scalar.copy(out=yb[r], in_=y)  # cast f32 -> fp16

            # ---- output projection (PE) ----
            for tt in range(NT):
                for oc in range(2):
                    ps = pp.tile([128, 512], F32, tag="pp", name="pp")
                    for r in range(NJ):
                        nc.tensor.matmul(
                            ps,
                            yb[r][:, tt * 128:(tt + 1) * 128],
                            wo_sb[:, r, oc * 512:(oc + 1) * 512],
                            start=(r == 0),
                            stop=(r == NJ - 1 and not with_bias),
                        )
                    if with_bias:
                        nc.tensor.matmul(
                            ps,
                            ones_sb[:, :128],
                            bo_sb[:, oc * 512:(oc + 1) * 512],
                            start=False,
                            stop=True,
                        )
                    o_sb = p_out.tile([128, 512], F32, tag="outs", name="osb")
                    nc.scalar.copy(out=o_sb, in_=ps)
                    nc.sync.dma_start(
                        out=out.ap()[t0 + tt * 128:t0 + (tt + 1) * 128,
                                     oc * 512:(oc + 1) * 512],
                        in_=o_sb,
                    )
    nc.compile()
    return nc


def _host_prep(query, key, Wq, Wk, Wo, bq, bk, bo):
    qT = np.ascontiguousarray(query.reshape(T_TOTAL, C).T).astype(NPDT)
    kT = np.ascontiguousarray(key.reshape(T_TOTAL, M, C).transpose(1, 2, 0)).astype(NPDT)

    wqT = np.ascontiguousarray(Wq.T).astype(NPDT)
    wkT = np.ascontiguousarray(Wk.T).astype(NPDT)
    woT = np.ascontiguousarray(Wo.T).astype(NPDT)

    j = np.arange(128)
    indl = np.stack([(j < 64), (j >= 64)], axis=1).astype(NPDT) * NPDT(SCALE)
    # indb[m, r, row, j] = 1 iff row == m*H + 2r + (j >= 64)
    rows = np.arange(64)[None, None, :, None]
    ms = np.arange(M)[:, None, None, None]
    rs = np.arange(NJ)[None, :, None, None]
    indb = (rows == ms * H + 2 * rs + (j[None, None, None, :] >= 64)).astype(NPDT)

    with_bias = bool(np.any(bq) or np.any(bk) or np.any(bo))
    common = {"wqT": wqT, "wkT": wkT, "woT": woT, "indl": indl, "indb": indb}
    if with_bias:
        common |= {
            "bq": bq.reshape(1, C).astype(NPDT),
            "bk": bk.reshape(1, C).astype(NPDT),
            "bo": bo.reshape(1, C).astype(NPDT),
        }
    in_maps = []
    for i in range(NCORES):
        sl = slice(i * T_CORE, (i + 1) * T_CORE)
        in_maps.append(
            {
                "qT": np.ascontiguousarray(qT[:, sl]),
                "kT": np.ascontiguousarray(kT[:, :, sl]),
                **common,
            }
        )
    return in_maps, with_bias


_NC_CACHE = {}
_LAST_RESULT = None


def kernel(query, key, gate, Wq, bq, Wk, bk, Wv, bv, Wg, bg, Wo, bo):
    in_maps, with_bias = _host_prep(query, key, Wq, Wk, Wo, bq, bk, bo)
    key_ = (T_CORE, with_bias)
    if key_ not in _NC_CACHE:
        _NC_CACHE[key_] = build_nc(T_CORE, with_bias)
    nc = _NC_CACHE[key_]
    res = run_bass_kernel_spmd(nc, in_maps, list(range(NCORES)))
    global _LAST_RESULT
    _LAST_RESULT = res
    out = np.concatenate([res.results[i]["out"] for i in range(NCORES)], axis=0)
    return out.reshape(B, N, C)



# revision 4
# speedup vs baseline: 1.2062x; 1.2062x over previous
"""Trainium2 Bass kernel for GatedCrossAttention (B=4, N=4096, C=1024, H=16, M=4).

Reference math (dead code removed: the v/gate projections are overwritten
by views of k in the original module, so v = g = k):
    q = query @ Wq.T + bq                    [B,N,C]   -> [B,N,H,hd]
    k = key   @ Wk.T + bk                    [B,N,M,C] -> [B,N,M,H,hd]
    attn = softmax_M(SCALE * einsum('bnhc,bnmhc->bnmh', q, k))
    out  = einsum('bnmh,bnmhc->bnhc', attn, k*k) . reshape(B,N,C)
    out  = out @ Wo.T + bo

Strategy: pure data parallel over the 16384 tokens (8 cores x 2048), no
collectives.  On-chip layout is "transposed": channels on partitions, tokens
on the free axis, so every matmul contraction is a natural PE op.

v2 rework (vs the 652us baseline):
  * Logits via a SHARED stationary indicator indl[128,16] per r-tile
    (col j = SCALE * 1[j == head(partition)]), moving operand = qp*kp.
    Lands logits head-major [16h, t] in PSUM with 8-way accumulation over
    r-tiles (each r contributes only its own two head rows, zeros
    elsewhere).  Replaces 128 tiny N=2 matmuls + PE transposes per block.
  * Softmax weights are broadcast back to channels with stationary
    indbT[16,128] matmuls whose emission is INTERLEAVED into the next
    block's projection r-loop, so the PE instruction stream never waits
    on the softmax chain (the baseline re-throttled the PE clock to
    1.2GHz every block through exactly that stall).
  * qp*kp and kp^2 are computed straight out of the projection PSUM banks
    (DVE mul / ScalarE Square), no kp SBUF copy.
  * m-reduction uses contiguous tree adds instead of a strided reduce.
Accumulation stays f32 in PSUM; activations/weights fp16 (rel err ~1e-3).
"""

import dataclasses
import numpy as np
from contextlib import ExitStack

try:
    import concourse.bass as bass
except ImportError:  # path fallback for bare containers
    import sys

    sys.path.insert(0, "/opt/trn_rl_repo")
    import concourse.bass as bass

import concourse.tile as tile
from concourse import bacc, mybir
from concourse.bass_utils import run_bass_kernel_spmd

# problem constants (hardcoded per the task contract)
B, N, C, H, HD, M = 4, 4096, 1024, 16, 64, 4
SCALE = float(HD) ** -0.5
NCORES = 8
T_TOTAL = B * N
T_CORE = T_TOTAL // NCORES  # 2048
TB = 256                    # tokens per block
NJ = C // 128               # 8 channel tiles (r/c run over these)
NT = TB // 128              # 2 token subtiles per block (for out proj)

DT = mybir.dt.float16
NPDT = np.float16
F32 = mybir.dt.float32
Exp = mybir.ActivationFunctionType.Exp


def _bcast(ap, reps, axis):
    """Insert a 0-stride dim of size `reps` at AP position `axis` (0=partition)."""
    new = list(ap.ap)
    new.insert(axis, [0, reps])
    return dataclasses.replace(ap, ap=new)


def build_nc(t_core=T_CORE, with_bias=False):
    nblk = t_core // TB
    nc = bacc.Bacc("TRN2", target_bir_lowering=False, debug=False)

    qT = nc.declare_dram_parameter("qT", [C, t_core], DT, isOutput=False)
    kT = nc.declare_dram_parameter("kT", [M, C, t_core], DT, isOutput=False)
    wqT = nc.declare_dram_parameter("wqT", [C, C], DT, isOutput=False)
    wkT = nc.declare_dram_parameter("wkT", [C, C], DT, isOutput=False)
    woT = nc.declare_dram_parameter("woT", [C, C], DT, isOutput=False)
    indl = nc.declare_dram_parameter("indl", [128, NJ, H], DT, isOutput=False)
    indb = nc.declare_dram_parameter("indb", [H, NJ, 128], DT, isOutput=False)
    if with_bias:
        bq = nc.declare_dram_parameter("bq", [1, C], DT, isOutput=False)
        bk = nc.declare_dram_parameter("bk", [1, C], DT, isOutput=False)
        bo = nc.declare_dram_parameter("bo", [1, C], DT, isOutput=False)
    out = nc.declare_dram_parameter("out", [t_core, C], F32, isOutput=True)

    # DRAM views: channel dim split into (chunk, partition)
    qT_v = qT.ap().rearrange("(c p) t -> p c t", p=128)
    kT_v = kT.ap().rearrange("m (c p) t -> p m c t", p=128)
    wq_v = wqT.ap().rearrange("(c p) j -> p c j", p=128)
    wk_v = wkT.ap().rearrange("(c p) j -> p c j", p=128)
    wo_v = woT.ap().rearrange("(c p) j -> p c j", p=128)

    with tile.TileContext(nc) as tc, ExitStack() as ctx:
        consts = ctx.enter_context(tc.tile_pool(name="consts", bufs=1))
        p_inq = ctx.enter_context(tc.tile_pool(name="inq", bufs=2))
        p_ink = ctx.enter_context(tc.tile_pool(name="ink", bufs=4))
        p_qp = ctx.enter_context(tc.tile_pool(name="qp", bufs=2))
        p_prod = ctx.enter_context(tc.tile_pool(name="prod", bufs=1))
        p_ksq = ctx.enter_context(tc.tile_pool(name="ksq", bufs=2))
        p_sm = ctx.enter_context(tc.tile_pool(name="sm", bufs=2))
        p_ct = ctx.enter_context(tc.tile_pool(name="ct", bufs=2))
        p_y = ctx.enter_context(tc.tile_pool(name="y", bufs=2))
        p_out = ctx.enter_context(tc.tile_pool(name="outs", bufs=4))
        # PSUM: pa (q-acc, logits-acc, outproj-acc) 3 banks,
        #       pb (k-acc, weight-broadcast) 5 banks
        pa = ctx.enter_context(tc.tile_pool(name="pa", bufs=3, space="PSUM"))
        pb = ctx.enter_context(tc.tile_pool(name="pb", bufs=5, space="PSUM"))

        # ---- constants / weights (resident) ----
        wq_sb = consts.tile([128, NJ, C], DT)
        wk_sb = consts.tile([128, NJ, C], DT)
        wo_sb = consts.tile([128, NJ, C], DT)
        nc.sync.dma_start(out=wq_sb, in_=wq_v)
        nc.sync.dma_start(out=wk_sb, in_=wk_v)
        nc.sync.dma_start(out=wo_sb, in_=wo_v)
        indl_sb = consts.tile([128, NJ, H], DT)
        indb_sb = consts.tile([H, NJ, 128], DT)
        nc.sync.dma_start(out=indl_sb, in_=indl.ap())
        nc.sync.dma_start(out=indb_sb, in_=indb.ap())
        if with_bias:
            ones_sb = consts.tile([1, TB], DT)
            nc.vector.memset(ones_sb, 1.0)
            bq_sb = consts.tile([1, C], DT)
            bk_sb = consts.tile([1, C], DT)
            bo_sb = consts.tile([1, C], DT)
            nc.sync.dma_start(out=bq_sb, in_=bq.ap())
            nc.sync.dma_start(out=bk_sb, in_=bk.ap())
            nc.sync.dma_start(out=bo_sb, in_=bo.ap())

        if not with_bias:
            ones_sb = bq_sb = bk_sb = bo_sb = None

        # cross-block pipeline state: (ksq, w_sb, y) of the previous block
        prev = None

        def emit_tail_mm(pv, r):
            """PE part of the prev-block tail for r-tile r (4 bcast MMs),
            plus the chasing DVE ops."""
            ksq_p, w_p, y_p = pv
            ctt = p_ct.tile([128, M, TB], DT, tag="ct", name="ctt")
            for m in range(M):
                wb = pb.tile([128, TB], F32, tag="pb", name="wb",
                             padded_shape=[128, 512])
                nc.tensor.matmul(wb, indb_sb[:, r, :], w_p[:, m, :],
                                 start=True, stop=True)
                nc.vector.tensor_mul(ctt[:, m, :], wb, ksq_p[:, r, m, :])
            a01 = p_ct.tile([128, TB], DT, tag="a01", name="a01")
            a23 = p_ct.tile([128, TB], DT, tag="a23", name="a23")
            nc.vector.tensor_add(a01, ctt[:, 0, :], ctt[:, 1, :])
            nc.vector.tensor_add(a23, ctt[:, 2, :], ctt[:, 3, :])
            nc.vector.tensor_add(y_p[:, r, :], a01, a23)

        for blk in range(nblk):
            t0 = blk * TB
            tsl = slice(t0, t0 + TB)

            # ---- load inputs (double buffered via pool bufs) ----
            q_in = p_inq.tile([128, NJ, TB], DT, tag="qin", name="qin")
            nc.sync.dma_start(out=q_in, in_=qT_v[:, :, tsl])
            k_in = [p_ink.tile([128, 2, NJ, TB], DT, tag="kin", name="kin")
                    for _ in range(2)]
            for pr in range(2):
                nc.sync.dma_start(
                    out=k_in[pr],
                    in_=kT_v[:, 2 * pr:2 * pr + 2, :, tsl],
                )

            qp = p_qp.tile([128, NJ, TB], DT, tag="qp", name="qp")
            prod = p_prod.tile([128, NJ, M, TB], DT, tag="prod", name="prod")
            ksq = p_ksq.tile([128, NJ, M, TB], DT, tag="ksq", name="ksq")
            y = p_y.tile([128, NJ, TB], DT, tag="y", name="y")

            # ---- projections (PE), prev-block weight-broadcast interleaved ----
            for r in range(NJ):
                q_ps = pa.tile([128, TB], F32, tag="pa", name="qps",
                               padded_shape=[128, 512])
                for c in range(NJ):
                    nc.tensor.matmul(
                        q_ps,
                        wq_sb[:, c, r * 128:(r + 1) * 128],
                        q_in[:, c, :],
                        start=(c == 0),
                        stop=(c == NJ - 1 and not with_bias),
                    )
                if with_bias:
                    nc.tensor.matmul(
                        q_ps, bq_sb[:, r * 128:(r + 1) * 128], ones_sb,
                        start=False, stop=True,
                    )
                nc.scalar.copy(out=qp[:, r, :], in_=q_ps)

                for m in range(M):
                    k_ps = pb.tile([128, TB], F32, tag="pb", name="kps",
                                   padded_shape=[128, 512])
                    for c in range(NJ):
                        nc.tensor.matmul(
                            k_ps,
                            wk_sb[:, c, r * 128:(r + 1) * 128],
                            k_in[m // 2][:, m % 2, c, :],
                            start=(c == 0),
                            stop=(c == NJ - 1 and not with_bias),
                        )
                    if with_bias:
                        nc.tensor.matmul(
                            k_ps, bk_sb[:, r * 128:(r + 1) * 128], ones_sb,
                            start=False, stop=True,
                        )
                    nc.vector.tensor_mul(prod[:, r, m, :], qp[:, r, :], k_ps)
                    nc.scalar.square(ksq[:, r, m, :], k_ps)

                # prev block's weight-broadcast + ct/y chase the proj stream
                if prev is not None:
                    emit_tail_mm(prev, r)

            # ---- attention logits, head-major: lg[h, t], accum over r ----
            e_sb = p_sm.tile([H, M, TB], F32, tag="e", name="e")
            for m in range(M):
                lg = pa.tile([H, TB], F32, tag="pa", name="lg",
                             padded_shape=[H, 512])
                for r in range(NJ):
                    nc.tensor.matmul(
                        lg,
                        indl_sb[:, r, :],
                        prod[:, r, m, :],
                        start=(r == 0),
                        stop=(r == NJ - 1),
                    )
                nc.scalar.activation(e_sb[:, m, :], lg, func=Exp)

            # ---- softmax over M (DVE, 16 partitions) ----
            s01 = p_sm.tile([H, TB], F32, tag="s01", name="s01")
            s23 = p_sm.tile([H, TB], F32, tag="s23", name="s23")
            s = p_sm.tile([H, TB], F32, tag="s", name="s")
            rcp = p_sm.tile([H, TB], F32, tag="rcp", name="rcp")
            w_sb = p_sm.tile([H, M, TB], DT, tag="w", name="w")
            nc.vector.tensor_add(s01, e_sb[:, 0, :], e_sb[:, 1, :])
            nc.vector.tensor_add(s23, e_sb[:, 2, :], e_sb[:, 3, :])
            nc.vector.tensor_add(s, s01, s23)
            nc.vector.reciprocal(rcp, s)
            nc.vector.tensor_mul(w_sb, e_sb, _bcast(rcp, M, 1))

            # ---- output projection of the PREVIOUS block (PE) ----
            if prev is not None:
                emit_outproj(nc, tc, prev, blk - 1, pa, p_out, wo_sb, out,
                             with_bias, ones_sb if with_bias else None,
                             bo_sb if with_bias else None)

            prev = (ksq, w_sb, y)

        # ---- drain: tail of the last block ----
        for r in range(NJ):
            emit_tail_mm(prev, r)
        emit_outproj(nc, tc, prev, nblk - 1, pa, p_out, wo_sb, out,
                     with_bias, ones_sb if with_bias else None,
                     bo_sb if with_bias else None)

    nc.compile()
    return nc


def emit_outproj(nc, tc, pv, blk, pa, p_out, wo_sb, out, with_bias, ones_sb,
                 bo_sb):
    _, _, y_p = pv
    t0 = blk * TB
    for tt in range(NT):
        for oc in range(2):
            o_ps = pa.tile([128, 512], mybir.dt.float32, tag="pa", name="ops")
            for r in range(NJ):
                nc.tensor.matmul(
                    o_ps,
                    y_p[:, r, tt * 128:(tt + 1) * 128],
                    wo_sb[:, r, oc * 512:(oc + 1) * 512],
                    start=(r == 0),
                    stop=(r == NJ - 1 and not with_bias),
                )
            if with_bias:
                nc.tensor.matmul(
                    o_ps,
                    ones_sb[:, :128],
                    bo_sb[:, oc * 512:(oc + 1) * 512],
                    start=False, stop=True,
                )
            o_sb = p_out.tile([128, 512], mybir.dt.float32, tag="outs",
                              name="osb")
            nc.scalar.copy(out=o_sb, in_=o_ps)
            nc.sync.dma_start(
                out=out.ap()[t0 + tt * 128:t0 + (tt + 1) * 128,
                             oc * 512:(oc + 1) * 512],
                in_=o_sb,
            )


def _host_prep(query, key, Wq, Wk, Wo, bq, bk, bo):
    query, key = np.asarray(query), np.asarray(key)
    Wq, Wk, Wo = np.asarray(Wq), np.asarray(Wk), np.asarray(Wo)
    bq, bk, bo = np.asarray(bq), np.asarray(bk), np.asarray(bo)
    qT = np.ascontiguousarray(query.reshape(T_TOTAL, C).T).astype(NPDT)
    kT = np.ascontiguousarray(key.reshape(T_TOTAL, M, C).transpose(1, 2, 0)).astype(NPDT)

    wqT = np.ascontiguousarray(Wq.T).astype(NPDT)
    wkT = np.ascontiguousarray(Wk.T).astype(NPDT)
    woT = np.ascontiguousarray(Wo.T).astype(NPDT)

    # indl[p, r, j] = SCALE * 1[j == 2r + (p>=64)] : logits indicator (lhsT)
    p = np.arange(128)[:, None, None]
    r = np.arange(NJ)[None, :, None]
    j = np.arange(H)[None, None, :]
    indl = (j == 2 * r + (p >= 64)).astype(NPDT) * NPDT(SCALE)
    # indb[j, r, p] = 1[j == 2r + (p>=64)] : broadcast indicator (lhsT)
    jj = np.arange(H)[:, None, None]
    rr = np.arange(NJ)[None, :, None]
    pp = np.arange(128)[None, None, :]
    indb = (jj == 2 * rr + (pp >= 64)).astype(NPDT)

    with_bias = bool(np.any(bq) or np.any(bk) or np.any(bo))
    common = {"wqT": wqT, "wkT": wkT, "woT": woT, "indl": indl, "indb": indb}
    if with_bias:
        common |= {
            "bq": bq.reshape(1, C).astype(NPDT),
            "bk": bk.reshape(1, C).astype(NPDT),
            "bo": bo.reshape(1, C).astype(NPDT),
        }
    in_maps = []
    for i in range(NCORES):
        sl = slice(i * T_CORE, (i + 1) * T_CORE)
        in_maps.append(
            {
                "qT": np.ascontiguousarray(qT[:, sl]),
                "kT": np.ascontiguousarray(kT[:, :, sl]),
                **common,
            }
        )
    return in_maps, with_bias


_NC_CACHE = {}
_LAST_RESULT = None


def kernel(query, key, gate, Wq, bq, Wk, bk, Wv, bv, Wg, bg, Wo, bo):
    in_maps, with_bias = _host_prep(query, key, Wq, Wk, Wo, bq, bk, bo)
    key_ = (T_CORE, with_bias)
    if key_ not in _NC_CACHE:
        _NC_CACHE[key_] = build_nc(T_CORE, with_bias)
    nc = _NC_CACHE[key_]
    res = run_bass_kernel_spmd(nc, in_maps, list(range(NCORES)))
    global _LAST_RESULT
    _LAST_RESULT = res
    out = np.concatenate([res.results[i]["out"] for i in range(NCORES)], axis=0)
    return out.reshape(B, N, C)


# revision 10
# speedup vs baseline: 1.4137x; 1.1721x over previous
"""Trainium2 Bass kernel for GatedCrossAttention (B=4, N=4096, C=1024, H=16, M=4).

Reference math (dead code removed: the v/gate projections are overwritten
by views of k in the original module, so v = g = k):
    q = query @ Wq.T + bq                    [B,N,C]   -> [B,N,H,hd]
    k = key   @ Wk.T + bk                    [B,N,M,C] -> [B,N,M,H,hd]
    attn = softmax_M(SCALE * einsum('bnhc,bnmhc->bnmh', q, k))
    out  = einsum('bnmh,bnmhc->bnhc', attn, k*k) . reshape(B,N,C)
    out  = out @ Wo.T + bo

Strategy: pure data parallel over the 16384 tokens (8 cores x 2048), no
collectives.  Channels live on partitions, tokens on the free axis, so every
matmul contraction is a natural PE op.

v3 (vs 652us baseline, 541us v2):
  * All matmuls N=512 moving (the per-MM overhead is ~24ns regardless of N,
    so wider is strictly better): k-proj streams an m-pair per MM, q-proj
    batches two token blocks per MM, logits/broadcast stream m-pairs.
  * Logits via a SHARED stationary indicator indl[128,16] per r-tile,
    landing logits head-major [16h, t] in PSUM with 8-way accumulation over
    r-tiles.  No transposes, no tiny-N matmuls.
  * Softmax weights return to channel-major via indbT[16,128] stationary
    matmuls interleaved into the NEXT block's projection loop, so the PE
    stream never waits on the softmax chain (keeps the HAM clock at 2.4GHz;
    the baseline oscillated to 1.2GHz every block).
  * qp*kp and kp^2 are computed straight out of the projection PSUM banks.
  * Weight DMAs are chunked per r-tile on a separate queue (gpsimd) from
    the input DMAs (sync), so the first matmul issues ~3us in instead of
    ~30us.
Accumulation stays f32 in PSUM; activations/weights fp16 (rel err ~1e-3).
"""

import dataclasses
import numpy as np
from contextlib import ExitStack

try:
    import concourse.bass as bass
except ImportError:  # path fallback for bare containers
    import sys

    sys.path.insert(0, "/opt/trn_rl_repo")
    import concourse.bass as bass

import concourse.tile as tile
from concourse import bacc, mybir
from concourse.bass_utils import run_bass_kernel_spmd

# problem constants (hardcoded per the task contract)
B, N, C, H, HD, M = 4, 4096, 1024, 16, 64, 4
SCALE = float(HD) ** -0.5
NCORES = 8
T_TOTAL = B * N
T_CORE = T_TOTAL // NCORES  # 2048
TB = 256                    # tokens per block
NJ = C // 128               # 8 channel tiles (r/c run over these)
NT = TB // 128              # 2 token subtiles per block (for out proj)

DT = mybir.dt.float16
NPDT = np.float16
F32 = mybir.dt.float32
Exp = mybir.ActivationFunctionType.Exp


def _bcast(ap, reps, axis):
    """Insert a 0-stride dim of size `reps` at AP position `axis` (0=partition)."""
    new = list(ap.ap)
    new.insert(axis, [0, reps])
    return dataclasses.replace(ap, ap=new)


def build_nc(t_core=T_CORE, with_bias=False):
    nblk = t_core // TB
    assert nblk % 2 == 0
    nc = bacc.Bacc("TRN2", target_bir_lowering=False, debug=False)

    qT = nc.declare_dram_parameter("qT", [C, t_core], DT, isOutput=False)
    # k pre-shuffled on host to [pair, c-chunk, m-in-pair, partition, t] so a
    # per-pair load is a 3-dim DMA landing [p, c, m, t] in SBUF
    kT = nc.declare_dram_parameter("kT", [2, NJ, 2, 128, t_core], DT,
                                   isOutput=False)
    wqT = nc.declare_dram_parameter("wqT", [C, C], DT, isOutput=False)
    wkT = nc.declare_dram_parameter("wkT", [C, C], DT, isOutput=False)
    woT = nc.declare_dram_parameter("woT", [C, C], DT, isOutput=False)
    indl = nc.declare_dram_parameter("indl", [128, NJ, H], DT, isOutput=False)
    indb = nc.declare_dram_parameter("indb", [H, NJ, 128], DT, isOutput=False)
    if with_bias:
        bq = nc.declare_dram_parameter("bq", [1, C], DT, isOutput=False)
        bk = nc.declare_dram_parameter("bk", [1, C], DT, isOutput=False)
        bo = nc.declare_dram_parameter("bo", [1, C], DT, isOutput=False)
    out = nc.declare_dram_parameter("out", [t_core, C], F32, isOutput=True)

    # DRAM views: channel dim split into (chunk, partition)
    qT_v = qT.ap().rearrange("(c p) t -> p c t", p=128)
    kT_v = kT.ap().rearrange("w c m p t -> w p c m t")
    wq_v = wqT.ap().rearrange("(c p) j -> p c j", p=128)
    wk_v = wkT.ap().rearrange("(c p) j -> p c j", p=128)
    wo_v = woT.ap().rearrange("(c p) j -> p c j", p=128)

    with tile.TileContext(nc) as tc, ExitStack() as ctx:
        consts = ctx.enter_context(tc.tile_pool(name="consts", bufs=1))
        p_inq = ctx.enter_context(tc.tile_pool(name="inq", bufs=2))
        p_ink = ctx.enter_context(tc.tile_pool(name="ink", bufs=4))
        p_qp = ctx.enter_context(tc.tile_pool(name="qp", bufs=2))
        p_prod = ctx.enter_context(tc.tile_pool(name="prod", bufs=1))
        p_ksq = ctx.enter_context(tc.tile_pool(name="ksq", bufs=2))
        p_sm = ctx.enter_context(tc.tile_pool(name="sm", bufs=2))
        p_ct = ctx.enter_context(tc.tile_pool(name="ct", bufs=2))
        p_y = ctx.enter_context(tc.tile_pool(name="y", bufs=2))
        p_out = ctx.enter_context(tc.tile_pool(name="outs", bufs=4))
        # PSUM (8 banks): pa = q-acc/logits-acc/outproj-acc, pk = k-acc,
        # pw = softmax-weight broadcast
        pa = ctx.enter_context(tc.tile_pool(name="pa", bufs=3, space="PSUM"))
        pk = ctx.enter_context(tc.tile_pool(name="pk", bufs=3, space="PSUM"))
        pw = ctx.enter_context(tc.tile_pool(name="pw", bufs=2, space="PSUM"))

        # ---- weights / constants: chunked per r-tile, on the gpsimd DMA
        # queue so they overlap the input DMAs on the sync queue ----
        wq_sb = consts.tile([128, NJ, C], DT)
        wk_sb = consts.tile([128, NJ, C], DT)
        wo_sb = consts.tile([128, NJ, C], DT)
        for r in range(NJ):
            js = slice(r * 128, (r + 1) * 128)
            nc.gpsimd.dma_start(out=wq_sb[:, :, js], in_=wq_v[:, :, js])
            nc.gpsimd.dma_start(out=wk_sb[:, :, js], in_=wk_v[:, :, js])
        indl_sb = consts.tile([128, NJ, H], DT)
        indb_sb = consts.tile([H, NJ, 128], DT)
        nc.gpsimd.dma_start(out=indl_sb, in_=indl.ap())
        nc.gpsimd.dma_start(out=indb_sb, in_=indb.ap())
        nc.gpsimd.dma_start(out=wo_sb, in_=wo_v)
        if with_bias:
            ones_sb = consts.tile([1, 2 * TB], DT)
            nc.vector.memset(ones_sb, 1.0)
            bq_sb = consts.tile([1, C], DT)
            bk_sb = consts.tile([1, C], DT)
            bo_sb = consts.tile([1, C], DT)
            nc.gpsimd.dma_start(out=bq_sb, in_=bq.ap())
            nc.gpsimd.dma_start(out=bk_sb, in_=bk.ap())
            nc.gpsimd.dma_start(out=bo_sb, in_=bo.ap())
        else:
            ones_sb = bq_sb = bk_sb = bo_sb = None

        # cross-block pipeline state: (ksq, w_sb, y) of the previous block
        prev = None
        qp = None
        q_in = None

        def emit_tail_mm(pv, r):
            """PE part of the prev-block tail for r-tile r (2 broadcast MMs
            of an m-pair each), plus the chasing DVE ops."""
            ksq_p, w_p, y_p = pv
            ctt = p_ct.tile([128, M, TB], DT, tag="ct", name="ctt")
            for pr in range(2):
                ms = slice(2 * pr, 2 * pr + 2)
                wb = pw.tile([128, 2, TB], F32, tag="pw", name="wb",
                             padded_shape=[128, 2, 256])
                nc.tensor.matmul(wb, indb_sb[:, r, :],
                                 w_p[:, ms, :], start=True, stop=True)
                nc.vector.tensor_mul(ctt[:, ms, :], wb, ksq_p[:, r, ms, :])
            a01 = p_ct.tile([128, TB], DT, tag="a01", name="a01")
            a23 = p_ct.tile([128, TB], DT, tag="a23", name="a23")
            nc.vector.tensor_add(a01, ctt[:, 0, :], ctt[:, 1, :])
            nc.vector.tensor_add(a23, ctt[:, 2, :], ctt[:, 3, :])
            nc.vector.tensor_add(y_p[:, r, :], a01, a23)

        def emit_outproj(pv, blk):
            _, _, y_p = pv
            t0 = blk * TB
            for tt in range(NT):
                for oc in range(2):
                    o_ps = pa.tile([128, 512], F32, tag="pa", name="ops")
                    for r in range(NJ):
                        nc.tensor.matmul(
                            o_ps,
                            y_p[:, r, tt * 128:(tt + 1) * 128],
                            wo_sb[:, r, oc * 512:(oc + 1) * 512],
                            start=(r == 0),
                            stop=(r == NJ - 1 and not with_bias),
                        )
                    if with_bias:
                        nc.tensor.matmul(
                            o_ps,
                            ones_sb[:, :128],
                            bo_sb[:, oc * 512:(oc + 1) * 512],
                            start=False, stop=True,
                        )
                    o_sb = p_out.tile([128, 512], F32, tag="outs", name="osb")
                    nc.scalar.copy(out=o_sb, in_=o_ps)
                    nc.sync.dma_start(
                        out=out.ap()[t0 + tt * 128:t0 + (tt + 1) * 128,
                                     oc * 512:(oc + 1) * 512],
                        in_=o_sb,
                    )

        for blk in range(nblk):
            t0 = blk * TB
            tsl = slice(t0, t0 + TB)
            par = blk % 2
            pton = slice(par * TB, (par + 1) * TB)

            # ---- input DMAs (sync queue, double buffered via pool bufs) ----
            if par == 0:
                q_in = p_inq.tile([128, NJ, 2 * TB], DT, tag="qin", name="qin")
                nc.sync.dma_start(out=q_in, in_=qT_v[:, :, t0:t0 + 2 * TB])
            k_in = [p_ink.tile([128, NJ, 2, TB], DT, tag="kin", name="kin")
                    for _ in range(2)]
            for pr in range(2):
                nc.sync.dma_start(out=k_in[pr], in_=kT_v[pr, :, :, :, tsl])

            if par == 0:
                qp = p_qp.tile([128, NJ, 2 * TB], DT, tag="qp", name="qp")
            prod = p_prod.tile([128, NJ, M, TB], DT, tag="prod", name="prod")
            ksq = p_ksq.tile([128, NJ, M, TB], DT, tag="ksq", name="ksq")
            y = p_y.tile([128, NJ, TB], DT, tag="y", name="y")

            # ---- projections (PE), prev-block tail interleaved per r ----
            for r in range(NJ):
                if par == 0:
                    # q for BOTH token blocks of this pair in one N=512 run
                    q_ps = pa.tile([128, 512], F32, tag="pa", name="qps")
                    for c in range(NJ):
                        nc.tensor.matmul(
                            q_ps,
                            wq_sb[:, c, r * 128:(r + 1) * 128],
                            q_in[:, c, :],
                            start=(c == 0),
                            stop=(c == NJ - 1 and not with_bias),
                        )
                    if with_bias:
                        nc.tensor.matmul(
                            q_ps, bq_sb[:, r * 128:(r + 1) * 128], ones_sb,
                            start=False, stop=True,
                        )
                    nc.scalar.copy(out=qp[:, r, :], in_=q_ps)

                for pr in range(2):
                    ms = slice(2 * pr, 2 * pr + 2)
                    k_ps = pk.tile([128, 2, TB], F32, tag="pk", name="kps",
                                   padded_shape=[128, 2, 256])
                    for c in range(NJ):
                        nc.tensor.matmul(
                            k_ps,
                            wk_sb[:, c, r * 128:(r + 1) * 128],
                            k_in[pr][:, c, :, :],
                            start=(c == 0),
                            stop=(c == NJ - 1 and not with_bias),
                        )
                    if with_bias:
                        nc.tensor.matmul(
                            k_ps, bk_sb[:, r * 128:(r + 1) * 128],
                            _bcast(ones_sb[:, :TB], 2, 1),
                            start=False, stop=True,
                        )
                    nc.vector.tensor_mul(
                        prod[:, r, ms, :],
                        _bcast(qp[:, r, pton], 2, 1), k_ps)
                    nc.scalar.square(ksq[:, r, ms, :], k_ps)

                # prev block's weight-broadcast + ct/y chase the proj stream
                if prev is not None:
                    emit_tail_mm(prev, r)

            # ---- attention logits, head-major [16h, t], accum over r ----
            e_sb = p_sm.tile([H, M, TB], F32, tag="e", name="e")
            for pr in range(2):
                ms = slice(2 * pr, 2 * pr + 2)
                lg = pa.tile([H, 2, TB], F32, tag="pa", name="lg",
                             padded_shape=[H, 2, 256])
                for r in range(NJ):
                    nc.tensor.matmul(
                        lg,
                        indl_sb[:, r, :],
                        prod[:, r, ms, :],
                        start=(r == 0),
                        stop=(r == NJ - 1),
                    )
                nc.scalar.activation(e_sb[:, ms, :], lg, func=Exp)

            # ---- softmax over M (DVE, 16 partitions) ----
            s01 = p_sm.tile([H, TB], F32, tag="s01", name="s01")
            s23 = p_sm.tile([H, TB], F32, tag="s23", name="s23")
            s = p_sm.tile([H, TB], F32, tag="s", name="s")
            rcp = p_sm.tile([H, TB], F32, tag="rcp", name="rcp")
            w_sb = p_sm.tile([H, M, TB], DT, tag="w", name="w")
            nc.vector.tensor_add(s01, e_sb[:, 0, :], e_sb[:, 1, :])
            nc.vector.tensor_add(s23, e_sb[:, 2, :], e_sb[:, 3, :])
            nc.vector.tensor_add(s, s01, s23)
            nc.vector.reciprocal(rcp, s)
            nc.vector.tensor_mul(w_sb, e_sb, _bcast(rcp, M, 1))

            # ---- output projection of the PREVIOUS block (PE) ----
            if prev is not None:
                emit_outproj(prev, blk - 1)

            prev = (ksq, w_sb, y)

        # ---- drain: tail of the last block ----
        for r in range(NJ):
            emit_tail_mm(prev, r)
        emit_outproj(prev, nblk - 1)

    nc.compile()
    return nc


def _host_prep(query, key, Wq, Wk, Wo, bq, bk, bo):
    query, key = np.asarray(query), np.asarray(key)
    Wq, Wk, Wo = np.asarray(Wq), np.asarray(Wk), np.asarray(Wo)
    bq, bk, bo = np.asarray(bq), np.asarray(bk), np.asarray(bo)
    qT = np.ascontiguousarray(query.reshape(T_TOTAL, C).T).astype(NPDT)
    # [pair, c-chunk, m-in-pair, partition, t]
    kT = (key.reshape(T_TOTAL, M, C).transpose(1, 2, 0)
          .reshape(2, 2, NJ, 128, T_TOTAL).transpose(0, 2, 1, 3, 4))
    kT = np.ascontiguousarray(kT).astype(NPDT)

    wqT = np.ascontiguousarray(Wq.T).astype(NPDT)
    wkT = np.ascontiguousarray(Wk.T).astype(NPDT)
    woT = np.ascontiguousarray(Wo.T).astype(NPDT)

    # indl[p, r, j] = SCALE * 1[j == 2r + (p>=64)] : logits indicator (lhsT)
    p = np.arange(128)[:, None, None]
    r = np.arange(NJ)[None, :, None]
    j = np.arange(H)[None, None, :]
    indl = (j == 2 * r + (p >= 64)).astype(NPDT) * NPDT(SCALE)
    # indb[j, r, p] = 1[j == 2r + (p>=64)] : broadcast indicator (lhsT)
    jj = np.arange(H)[:, None, None]
    rr = np.arange(NJ)[None, :, None]
    pp = np.arange(128)[None, None, :]
    indb = (jj == 2 * rr + (pp >= 64)).astype(NPDT)

    with_bias = bool(np.any(bq) or np.any(bk) or np.any(bo))
    common = {"wqT": wqT, "wkT": wkT, "woT": woT, "indl": indl, "indb": indb}
    if with_bias:
        common |= {
            "bq": bq.reshape(1, C).astype(NPDT),
            "bk": bk.reshape(1, C).astype(NPDT),
            "bo": bo.reshape(1, C).astype(NPDT),
        }
    in_maps = []
    for i in range(NCORES):
        sl = slice(i * T_CORE, (i + 1) * T_CORE)
        in_maps.append(
            {
                "qT": np.ascontiguousarray(qT[:, sl]),
                "kT": np.ascontiguousarray(kT[:, :, :, :, sl]),
                **common,
            }
        )
    return in_maps, with_bias


_NC_CACHE = {}
_LAST_RESULT = None


def kernel(query, key, gate, Wq, bq, Wk, bk, Wv, bv, Wg, bg, Wo, bo):
    in_maps, with_bias = _host_prep(query, key, Wq, Wk, Wo, bq, bk, bo)
    key_ = (T_CORE, with_bias)
    if key_ not in _NC_CACHE:
        _NC_CACHE[key_] = build_nc(T_CORE, with_bias)
    nc = _NC_CACHE[key_]
    res = run_bass_kernel_spmd(nc, in_maps, list(range(NCORES)))
    global _LAST_RESULT
    _LAST_RESULT = res
    out = np.concatenate([res.results[i]["out"] for i in range(NCORES)], axis=0)
    return out.reshape(B, N, C)
